# revision 20
# baseline (speedup 1.0000x reference)
"""GNN message-passing edge scorer on 8 TRN2 NeuronCores.

Model: out[e] = relu(concat(U[src[e]], M[dst[e]]) @ W1 + b1) @ W2 + b2
  U, M: [100000, 128] f32 node tables; edge_index: [2, 1000000] int32/64.

v3 strategy (edge-parallel, tables replicated; algebraic restructure):
  W2 is folded into the tables: with g = |w2| * (a_src + b_dst) in a
  feature order permuted so positive-sign w2 features come first,
    out[e] = sum_{f<PP} relu(g_f) - sum_{f>=PP} relu(g_f) + b2.
  - NEFF-A (8-core SPMD): each core computes its 1/8 shard of the
    PRE-MIXED+SCALED tables A = U @ (W1top |w2| perm) and
    B' = M @ (W1bot |w2| perm) + b1', written node-major fp16.
  - NEFF-B (8-core SPMD): edges sharded by dst; B' rows expanded from the
    core's SBUF-resident slab by one-hot S-matrix matmuls; A rows pulled
    by dma_gather in BIG (4096-row) calls -- cells are ordered
    chunk-major so calls merge across (chunk, window) cells.
    Per 4-quarter group: 4x(mm1 S-expand + mm2 gathered-add) on PE,
    one ACT relu (PSUM->SBUF), two DVE range-split tensor_reduces.
    No per-edge vector multiply remains.

HW facts this design is built on (measured on this runtime):
  - dma_gather descriptor path: ~1.9-2.3 ns/row with 4 SWDGE queues and
    4096-row calls (vs ~2.9 at 512-row calls, ~3.5 at ~384); descriptor
    cost is per ROW (independent of elem size).
  - num_idxs_reg truncation and trailing -1 indices HANG the device
    (semaphore mismatch) => padded slots must really be gathered.
  - indirect_dma_start: ~11 ns/row -- not competitive.
  - Plain/contiguous DMA runs ~190-360 GB/s: dense table precompute and
    the one-hot S stream are cheap vs per-edge gathers.
"""

import numpy as np

N_NODES = 100000
H = 128
N_CORES = 8
SHARD = 12544                 # NEFF-A rows per core (98 * 128)
NB_A = SHARD // 128           # 98 blocks per table per core
NP = SHARD * N_CORES          # 100352 padded table rows
N_CHUNKS = 4
CHUNK = NP // N_CHUNKS        # 25088, int16-addressable
W_SZ = 128                    # dst window rows
N_WIN = SHARD // W_SZ         # 98 windows per core
MAX_CALL = 8192               # indices per dma_gather call
N_QUEUES = 4                  # SWDGE queues for gathers
GRP = 4                       # quarters per compute group
S_GRP = 16                    # S-matrix quarters per streaming DMA

_cache = {}


def _build_neff_a(reps=1):
    """Table precompute: A = U @ W1top', B = M @ W1bot' + b1'.

    Inputs are host-TRANSPOSED fp16 (ushT[:, nb*128+p] = U[p*NB_A+nb]) so
    each table is 25 big constant-stationary matmuls; the ACT evacuation
    fuses the b1 bias; one xbar transpose restores node-major rows.
    """
    import concourse.bacc as bacc
    import concourse.mybir as mybir
    import concourse.tile as tile

    f32 = mybir.dt.float32
    fp16 = mybir.dt.float16
    ACT = mybir.ActivationFunctionType

    BLK = 512
    blocks = [(i * BLK, min(BLK, SHARD - i * BLK))
              for i in range(-(-SHARD // BLK))]

    nc = bacc.Bacc("TRN2", target_bir_lowering=False, debug=False,
                   num_devices=N_CORES)
    ushT = nc.dram_tensor("ushT", [128, SHARD], fp16, kind="ExternalInput")
    mshT = nc.dram_tensor("mshT", [128, SHARD], fp16, kind="ExternalInput")
    wk = nc.dram_tensor("wk", [128, 2 * H], fp16, kind="ExternalInput")
    b1c = nc.dram_tensor("b1c", [128, 1], f32, kind="ExternalInput")
    a16o = nc.dram_tensor("a16o", [SHARD, H], fp16, kind="ExternalOutput")
    b16o = nc.dram_tensor("b16o", [SHARD, H], fp16, kind="ExternalOutput")

    with tile.TileContext(nc) as tc:
        with (
            tc.tile_pool(name="src", bufs=2) as spool,
            tc.tile_pool(name="at", bufs=2) as atpool,
            tc.tile_pool(name="stg", bufs=2) as stpool,
            tc.tile_pool(name="w", bufs=1) as wpool,
            tc.tile_pool(name="pa", bufs=4, space="PSUM") as papool,
        ):
            wsb = wpool.tile([128, 2 * H], fp16, tag="wsb")
            nc.sync.dma_start(wsb[:], wk[:])
            bsb = wpool.tile([128, 1], f32, tag="bsb")
            nc.sync.dma_start(bsb[:], b1c[:])

            def one_table(src_d, out_d, wcol, is_b):
                usbT = spool.tile([128, SHARD], fp16, tag="usbT")
                nc.sync.dma_start(usbT[:], src_d[:])
                atT = atpool.tile([128, SHARD], fp16, tag="atT")
                for c0, n in blocks:
                    pa = papool.tile([128, BLK], f32, tag="pa")
                    nc.tensor.matmul(pa[:, :n], wsb[:, wcol * H:(wcol + 1) * H],
                                     usbT[:, c0:c0 + n], start=True, stop=True)
                    if is_b:
                        nc.scalar.activation(atT[:, c0:c0 + n], pa[:, :n],
                                             ACT.Identity, bias=bsb[:],
                                             scale=1.0)
                    else:
                        nc.scalar.activation(atT[:, c0:c0 + n], pa[:, :n],
                                             ACT.Copy)
                stg = stpool.tile([128, NB_A, 128], fp16, tag="stg")
                nc.sync.dma_start_transpose(stg[:], atT[:])
                nc.sync.dma_start(out_d[:], stg[:])

            def body():
                one_table(ushT, a16o, 0, False)
                one_table(mshT, b16o, 1, True)

            body()
            if reps > 1:
                with tc.For_i(0, reps - 1):
                    body()
    nc.compile()
    return nc


def _build_neff_b3(q_cell, pp, reps=1, ablate=()):
    """Chunk-major windowed NEFF with big merged gathers.

    q_cell: tuple of N_CHUNKS*N_WIN ints -- 128-slot quarters per
    (chunk a, window w) cell in a-major order, shared across cores.
    pp: feature split point (positive-sign w2 features first).
    Output: out[s % 128, s // 128] for padded slot s.
    """
    import concourse.bacc as bacc
    import concourse.mybir as mybir
    import concourse.tile as tile

    f32 = mybir.dt.float32
    fp16 = mybir.dt.float16
    i16 = mybir.dt.int16
    ACT = mybir.ActivationFunctionType
    ALU = mybir.AluOpType

    n_q = sum(q_cell)
    s_tot = n_q * 128
    out_cols = n_q

    # window of each quarter (a-major cell order)
    q_win = []
    for ab, q in enumerate(q_cell):
        w = ab % N_WIN
        q_win.extend([w] * q)

    # gather calls: per chunk, quarters merged into <=MAX_CALL-row calls
    chunk_q = [sum(q_cell[a * N_WIN:(a + 1) * N_WIN]) for a in range(N_CHUNKS)]
    calls = []                    # (chunk a, slot0, n_rows)
    q2call = []                   # quarter -> (call idx, slice within call)
    qbase = 0
    for a in range(N_CHUNKS):
        nq_a = chunk_q[a]
        done = 0
        while done < nq_a:
            take = min(MAX_CALL // 128, nq_a - done)
            ci = len(calls)
            calls.append((a, (qbase + done) * 128, take * 128))
            for k in range(take):
                q2call.append((ci, k))
            done += take
        qbase += nq_a
    assert len(q2call) == n_q

    nc = bacc.Bacc("TRN2", target_bir_lowering=False, debug=False,
                   num_devices=N_CORES, num_swdge_queues=N_QUEUES)
    a16 = nc.dram_tensor("a16", [NP, H], fp16, kind="ExternalInput")
    bslab = nc.dram_tensor("bslab", [128, N_WIN * H], fp16, kind="ExternalInput")
    uidx = nc.dram_tensor("uidx", [128, s_tot // 16], i16, kind="ExternalInput")
    dstw = nc.dram_tensor("dstw", [1, s_tot], fp16, kind="ExternalInput")
    wp2 = nc.dram_tensor("wp2", [128, H + 2], f32, kind="ExternalInput")
    out = nc.dram_tensor("out", [128, out_cols], f32, kind="ExternalOutput")

    with tile.TileContext(nc) as tc:
        with (
            tc.tile_pool(name="g", bufs=4) as gpool,
            tc.tile_pool(name="b", bufs=3) as bpool,
            tc.tile_pool(name="s", bufs=3) as spool,
            tc.tile_pool(name="h", bufs=4) as hpool,
            tc.tile_pool(name="ps", bufs=3, space="PSUM") as pspool,
            tc.tile_pool(name="w", bufs=1) as wpool,
            tc.tile_pool(name="o", bufs=1) as opool,
            tc.tile_pool(name="ix", bufs=1) as idxp,
            tc.tile_pool(name="slab", bufs=1) as slabp,
        ):
            uix = idxp.tile([128, s_tot // 16], i16, tag="uix")
            nc.sync.dma_start(uix[:], uidx[:])
            slab = slabp.tile([128, N_WIN * H], fp16, tag="slab")
            nc.sync.dma_start(slab[:], bslab[:])
            wsb = wpool.tile([128, H + 2], f32, tag="wsb")
            nc.sync.dma_start(wsb[:], wp2[:])
            id16 = wpool.tile([128, H], fp16, tag="id16")
            nc.scalar.activation(id16[:], wsb[:, 0:H], ACT.Copy)
            iota = wpool.tile([128, 1], fp16, tag="iota")
            nc.scalar.activation(iota[:], wsb[:, H:H + 1], ACT.Copy)
            b2c = wsb[:, H + 1:H + 2]
            o_rp = opool.tile([128, out_cols], f32, tag="orp")
            o_rm = opool.tile([128, out_cols], f32, tag="orm")
            o_sb = opool.tile([128, out_cols], f32, tag="osb")
            if pp == 128:
                nc.vector.memset(o_rm[:], 0.0)
            if pp == 0:
                nc.vector.memset(o_rp[:], 0.0)

            qctr = [0]

            def body():
                gtiles = [None] * len(calls)
                stile = [None]

                def s_quarter(k):
                    if k % S_GRP == 0:
                        w = min(S_GRP, n_q - k) * 128
                        bco = bpool.tile([128, S_GRP * 128], fp16, tag="b")
                        nc.sync.dma_start(
                            bco[:, :w],
                            dstw[0:1, k * 128:k * 128 + w]
                            .to_broadcast([128, w]))
                        if "iseq" in ablate:
                            return None
                        stile[0] = spool.tile([128, S_GRP, 128], fp16, tag="s",
                                              name="stile")
                        nc.vector.tensor_tensor(
                            stile[0][:, :w // 128, :], bco[:, :w],
                            iota[:].to_broadcast([128, w]),
                            op=ALU.is_equal)
                    if "iseq" in ablate:
                        return None
                    return stile[0][:, k % S_GRP, :]

                def issue_call(ci):
                    if "gather" in ablate:
                        gtiles[ci] = True
                        return
                    a, s0, n = calls[ci]
                    gt = gpool.tile([128, MAX_CALL // 128, H], fp16, tag="ug")
                    nc.gpsimd.dma_gather(
                        gt[:, :n // 128, :], a16[a * CHUNK:(a + 1) * CHUNK, :],
                        uix[:, s0 // 16:(s0 + n) // 16],
                        num_idxs=n, num_idxs_reg=n, elem_size=H,
                        transpose=False, single_packet=False,
                        queue_num=qctr[0] % N_QUEUES)
                    qctr[0] += 1
                    gtiles[ci] = gt

                if "compute" in ablate:
                    for ci in range(len(calls)):
                        issue_call(ci)
                for g0 in range(0, n_q, GRP):
                    if "compute" in ablate:
                        break
                    gn = min(GRP, n_q - g0)
                    # make sure gather tiles for this group's quarters exist
                    for j in range(gn):
                        ci, _ = q2call[g0 + j]
                        if gtiles[ci] is None:
                            issue_call(ci)
                    # prefetch ahead (keeps queues busy)
                    ci_last = q2call[g0 + gn - 1][0]
                    for ahead in (1, 2):
                        cn = ci_last + ahead
                        if cn < len(calls) and gtiles[cn] is None:
                            issue_call(cn)
                    ps = pspool.tile([128, GRP, H], f32, tag="ps")
                    for j in range(gn):
                        q = g0 + j
                        if "mm" in ablate:
                            if "sbuild" not in ablate:
                                s_quarter(q)
                            continue
                        s_ap = s_quarter(q)
                        mini = slab[:, q_win[q] * H:(q_win[q] + 1) * H]
                        nc.tensor.matmul(ps[:, j, :], s_ap, mini,
                                         start=True, stop=False)
                        ci, k = q2call[q]
                        rhs2 = (id16[:] if "gather" in ablate
                                else gtiles[ci][:, k, :])
                        nc.tensor.matmul(ps[:, j, :], id16[:], rhs2,
                                         start=False, stop=True)
                    if "mm" in ablate or "act" in ablate:
                        continue
                    # split relu outputs into two CONTIGUOUS tiles so the
                    # DVE reduces run unstrided at full rate
                    hl = hpool.tile([128, GRP, pp], fp16, tag="hl")
                    hr = hpool.tile([128, GRP, 128 - pp], fp16, tag="hr")
                    if pp > 0:
                        nc.scalar.activation(hl[:, :gn, :], ps[:, :gn, 0:pp],
                                             ACT.Relu)
                    if pp < 128:
                        nc.scalar.activation(hr[:, :gn, :], ps[:, :gn, pp:128],
                                             ACT.Relu)
                    if "reduce" in ablate or "act" in ablate:
                        continue
                    if pp > 0:
                        nc.vector.tensor_reduce(
                            o_rp[:, g0:g0 + gn], hl[:, :gn, :],
                            axis=mybir.AxisListType.X, op=ALU.add)
                    if pp < 128:
                        nc.vector.tensor_reduce(
                            o_rm[:, g0:g0 + gn], hr[:, :gn, :],
                            axis=mybir.AxisListType.X, op=ALU.add)
                # out = rp - rm + b2
                if not ablate:
                    nc.vector.tensor_tensor(o_sb[:], o_rp[:], o_rm[:],
                                            op=ALU.subtract)
                    nc.scalar.activation(o_sb[:], o_sb[:], ACT.Identity,
                                         bias=b2c, scale=1.0)
                else:
                    nc.vector.memset(o_sb[:], 0.0)

            body()
            if reps > 1:
                with tc.For_i(0, reps - 1):
                    body()
            nc.sync.dma_start(out[:], o_sb[:])
    nc.compile()
    return nc, out_cols


def _marshal3(edge_index):
    """dst-sharded, chunk-major (a, w) cell marshalling.

    Core c owns dst rows [c*SHARD, (c+1)*SHARD); its edges are grouped by
    (chunk a = src // CHUNK, window w = (dst % SHARD) // 128) cells in
    a-major order, padded per cell to q_cell[a,w]*128 slots (q_cell
    shared across cores).  Returns q_cell and per-core uidx/sin/inv.
    """
    src = np.asarray(edge_index[0]).astype(np.int64)
    dst = np.asarray(edge_index[1]).astype(np.int64)
    core_of = dst // SHARD
    a_of = src // CHUNK
    n_cells = N_CHUNKS * N_WIN

    # Balanced-window packing: assign each core's 12544 dst rows to its 98
    # windows so per-(chunk, window) edge counts pack tightly under a shared
    # cap grid (minimizing 128-slot padding).  win_of[c] maps local dst row
    # -> window.
    win_of = np.empty((N_CORES, SHARD), dtype=np.int64)
    percell = np.zeros((N_CORES, N_CHUNKS, N_WIN), dtype=np.int64)
    # shared cap grid: X windows at 3 quarters, rest at 2 (per chunk)
    demand = np.zeros((N_CORES, N_CHUNKS), dtype=np.int64)
    rowcnts = []
    for c in range(N_CORES):
        m = core_of == c
        loc = dst[m] - c * SHARD
        rowcnt = np.zeros((SHARD, N_CHUNKS), dtype=np.int64)
        np.add.at(rowcnt, (loc, a_of[m]), 1)
        rowcnts.append(rowcnt)
        demand[c] = rowcnt.sum(axis=0)
    dmax = int(demand.max())
    X = min(N_WIN, max(0, -(-(dmax + 500 - 254 * N_WIN) // 128)))
    for c in range(N_CORES):
        rowcnt = rowcnts[c]
        tot = rowcnt.sum(axis=1)
        order = np.argsort(-tot, kind="stable")
        caps = np.full((N_WIN, N_CHUNKS), 3 * 128 - 1, dtype=np.int64)
        caps[X:] = 2 * 128 - 1
        room = np.full(N_WIN, W_SZ, dtype=np.int64)
        used = np.zeros((N_WIN, N_CHUNKS), dtype=np.int64)
        for r in order:
            v = rowcnt[r]
            ok = (room > 0) & np.all(used + v <= caps, axis=1)
            if ok.any():
                cand = np.nonzero(ok)[0]
                # best fit: window whose max normalized load after placing
                # is largest (fills tight windows first) with room spread
                resid = ((used[cand] + v) / caps[cand]).max(axis=1)
                w = cand[np.argmax(resid - 0.001 * room[cand])]
            else:  # overflow: window with most per-chunk headroom
                cand = np.nonzero(room > 0)[0]
                w = cand[np.argmax((caps[cand] - used[cand] - v).min(axis=1))]
            win_of[c, r] = w
            used[w] += v
            room[w] -= 1
        percell[c] = used.T
    cnt = percell.reshape(N_CORES, n_cells)
    q_cell = tuple(int(x) for x in -(-cnt.max(axis=0) // 128))

    # fall back to the plain row->window layout if packing did not win
    plain_w = np.tile((np.arange(SHARD) // W_SZ), (N_CORES, 1))
    cnt_pl = np.zeros((N_CORES, n_cells), dtype=np.int64)
    for c in range(N_CORES):
        m = core_of == c
        cell_pl = a_of[m] * N_WIN + plain_w[c, dst[m] - c * SHARD]
        cnt_pl[c] = np.bincount(cell_pl, minlength=n_cells)
    q_pl = tuple(int(x) for x in -(-cnt_pl.max(axis=0) // 128))
    if sum(q_pl) <= sum(q_cell):
        win_of = plain_w
        cnt = cnt_pl
        q_cell = q_pl

    w_of = win_of[core_of, dst - core_of * SHARD]
    cell_of = a_of * N_WIN + w_of
    n_q = int(sum(q_cell))
    s_tot = n_q * 128
    cell_base = np.concatenate([[0], np.cumsum(np.asarray(q_cell) * 128)])

    cores = []
    for c in range(N_CORES):
        m = np.nonzero(core_of == c)[0]
        order = m[np.argsort(cell_of[m], kind="stable")]
        cells_sorted = cell_of[order]
        starts = np.searchsorted(cells_sorted, np.arange(n_cells), side="left")
        within = np.arange(order.size) - starts[cells_sorted]
        slots = cell_base[cells_sorted] + within

        # pad slots get spread indices -- a constant (eg row 0) makes every
        # padded descriptor hit one HBM address and halves gather throughput
        uloc = (np.arange(s_tot, dtype=np.int64) * 97 % CHUNK).astype(np.int16)
        uloc[slots] = (src[order] - a_of[order] * CHUNK).astype(np.int16)
        # rank of each dst row within its window (slab row index)
        lw = win_of[c]
        rank = np.zeros(SHARD, dtype=np.int64)
        ws = np.argsort(lw, kind="stable")
        rank[ws] = np.arange(SHARD) - np.searchsorted(lw[ws], lw[ws]) * 0
        starts_w = np.searchsorted(lw[ws], np.arange(N_WIN))
        rank[ws] = np.arange(SHARD) - starts_w[lw[ws]]
        dstrow = np.full(s_tot, -1, dtype=np.int64)
        dstrow[slots] = rank[dst[order] - c * SHARD]
        inv = np.full(s_tot, -1, dtype=np.int64)
        inv[slots] = order

        dstw = np.ascontiguousarray(
            dstrow.astype(np.float16).reshape(1, s_tot))

        wrapped = np.ascontiguousarray(
            np.tile(uloc.reshape(s_tot // 16, 16).T, (8, 1)))
        perm = np.argsort(win_of[c] * SHARD + rank, kind="stable")
        cores.append({"uidx": wrapped, "dstw": dstw, "inv": inv,
                      "rowperm": perm})
    return q_cell, n_q, cores


def _prep_wp2(W2, b2):
    wp2 = np.zeros((128, H + 2), dtype=np.float32)
    wp2[:, 0:H] = np.eye(128, dtype=np.float32)
    wp2[:, H] = np.arange(128, dtype=np.float32)
    wp2[:, H + 1] = np.asarray(b2, dtype=np.float32)[0]
    return wp2


def _slab_for_core(B16, c, rowperm=None):
    rows = B16[c * SHARD:(c + 1) * SHARD]
    if rowperm is not None:
        rows = rows[rowperm]
    return np.ascontiguousarray(
        rows.reshape(N_WIN, W_SZ, H).transpose(1, 0, 2).reshape(128, N_WIN * H))


def _fold_w2(W1, b1, W2):
    """Fold |w2| scaling + positive-first sign permutation into W1/b1."""
    W1 = np.asarray(W1, dtype=np.float32)
    b1 = np.asarray(b1, dtype=np.float32)
    w2 = np.asarray(W2, dtype=np.float32).reshape(H)
    sign_neg = w2 < 0
    perm = np.argsort(sign_neg, kind="stable")  # positives (and 0) first
    pp = int((~sign_neg).sum())
    scale = np.abs(w2)
    W1p = (W1 * scale[None, :])[:, perm]
    b1p = (b1 * scale)[perm]
    return W1p, b1p, pp


def _prep_a_inputs(user_features, movie_features, W1, b1, W2=None):
    if W2 is not None:
        W1, b1, _ = _fold_w2(W1, b1, W2)
    uf = np.zeros((NP, H), dtype=np.float16)
    uf[:N_NODES] = user_features.astype(np.float16)
    mf = np.zeros((NP, H), dtype=np.float16)
    mf[:N_NODES] = movie_features.astype(np.float16)
    wk = np.zeros((128, 2 * H), dtype=np.float16)
    wk[:, 0:H] = np.asarray(W1, dtype=np.float32)[:H].astype(np.float16)
    wk[:, H:2 * H] = np.asarray(W1, dtype=np.float32)[H:].astype(np.float16)
    b1c = np.ascontiguousarray(
        np.asarray(b1, dtype=np.float32).reshape(128, 1))
    j = np.arange(SHARD)
    rowmap = (j % 128) * NB_A + j // 128
    outs = []
    for c in range(N_CORES):
        us = uf[c * SHARD:(c + 1) * SHARD]
        ms = mf[c * SHARD:(c + 1) * SHARD]
        outs.append({"ushT": np.ascontiguousarray(us[rowmap].T),
                     "mshT": np.ascontiguousarray(ms[rowmap].T),
                     "wk": wk, "b1c": b1c})
    return outs


def kernel(user_features, movie_features, edge_index, W1, b1, W2, b2):
    from concourse.bass_utils import run_bass_kernel_spmd

    user_features = np.ascontiguousarray(user_features, dtype=np.float32)
    movie_features = np.ascontiguousarray(movie_features, dtype=np.float32)
    ei = np.ascontiguousarray(edge_index)
    E = ei.shape[1]

    W1p, b1p, pp = _fold_w2(W1, b1, W2)

    # ---- NEFF-A: device-side A = U@W1top', B' = M@W1bot' + b1' (fp16) ----
    if "A" not in _cache:
        _cache["A"] = _build_neff_a()
    nca = _cache["A"]
    in_a = _prep_a_inputs(user_features, movie_features, W1p, b1p)
    res_a = run_bass_kernel_spmd(nca, in_a, core_ids=list(range(N_CORES)))
    A16 = np.concatenate([res_a.results[c]["a16o"] for c in range(N_CORES)])
    B16 = np.concatenate([res_a.results[c]["b16o"] for c in range(N_CORES)])

    # ---- host marshalling of edges (chunk-major windowed cells) ----
    q_cell, n_q, cores = _marshal3(ei)

    key_b = ("B3", q_cell, pp)
    if key_b not in _cache:
        _cache[key_b] = _build_neff_b3(q_cell, pp)
    ncb, out_cols = _cache[key_b]

    wp2 = _prep_wp2(W2, b2)
    in_b = [{"a16": A16,
             "bslab": _slab_for_core(B16, c, cores[c]["rowperm"]),
             "uidx": cores[c]["uidx"], "dstw": cores[c]["dstw"], "wp2": wp2}
            for c in range(N_CORES)]
    res_b = run_bass_kernel_spmd(ncb, in_b, core_ids=list(range(N_CORES)))

    # ---- host inverse permutation ----
    # padded-stream slot s lives at device out[s % 128, s // 128]
    out = np.empty(E, dtype=np.float32)
    s = np.arange(n_q * 128)
    flat_pos = (s % 128) * out_cols + s // 128
    for c in range(N_CORES):
        vals = res_b.results[c]["out"].reshape(-1)[flat_pos]
        inv = cores[c]["inv"]
        mask = inv >= 0
        out[inv[mask]] = vals[mask]
    return out


# revision 21
# speedup vs baseline: 1.2750x; 1.2750x over previous
"""GNN message-passing edge scorer on 8 TRN2 NeuronCores.

Model: out[e] = relu(concat(U[src[e]], M[dst[e]]) @ W1 + b1) @ W2 + b2
  U, M: [100000, 128] f32 node tables; edge_index: [2, 1000000] int32/64.

v3 strategy (edge-parallel, tables replicated; algebraic restructure):
  W2 is folded into the tables: with g = |w2| * (a_src + b_dst) in a
  feature order permuted so positive-sign w2 features come first,
    out[e] = sum_{f<PP} relu(g_f) - sum_{f>=PP} relu(g_f) + b2.
  - NEFF-A (8-core SPMD): each core computes its 1/8 shard of the
    PRE-MIXED+SCALED tables A = U @ (W1top |w2| perm) and
    B' = M @ (W1bot |w2| perm) + b1', written node-major fp16.
  - NEFF-B (8-core SPMD): edges sharded by dst; B' rows expanded from the
    core's SBUF-resident slab by one-hot S-matrix matmuls; A rows pulled
    by dma_gather in BIG (4096-row) calls -- cells are ordered
    chunk-major so calls merge across (chunk, window) cells.
    Per 4-quarter group: 4x(mm1 S-expand + mm2 gathered-add) on PE,
    one ACT relu (PSUM->SBUF), two DVE range-split tensor_reduces.
    No per-edge vector multiply remains.

HW facts this design is built on (measured on this runtime):
  - dma_gather descriptor path: ~1.9-2.3 ns/row with 4 SWDGE queues and
    4096-row calls (vs ~2.9 at 512-row calls, ~3.5 at ~384); descriptor
    cost is per ROW (independent of elem size).
  - num_idxs_reg truncation and trailing -1 indices HANG the device
    (semaphore mismatch) => padded slots must really be gathered.
  - indirect_dma_start: ~11 ns/row -- not competitive.
  - Plain/contiguous DMA runs ~190-360 GB/s: dense table precompute and
    the one-hot S stream are cheap vs per-edge gathers.
"""

import numpy as np

N_NODES = 100000
H = 128
N_CORES = 8
SHARD = 12544                 # NEFF-A rows per core (98 * 128)
NB_A = SHARD // 128           # 98 blocks per table per core
NP = SHARD * N_CORES          # 100352 padded table rows
N_CHUNKS = 4
CHUNK = NP // N_CHUNKS        # 25088, int16-addressable
W_SZ = 128                    # dst window rows
N_WIN = SHARD // W_SZ         # 98 windows per core
MAX_CALL = 4096               # indices per dma_gather call
N_QUEUES = 4                  # SWDGE queues for gathers
GRP = 4                       # quarters per compute group
S_GRP = 16                    # S-matrix quarters per streaming DMA

_cache = {}


def _build_neff_a(reps=1):
    """Table precompute: A = U @ W1top', B = M @ W1bot' + b1'.

    Inputs are host-TRANSPOSED fp16 (ushT[:, nb*128+p] = U[p*NB_A+nb]) so
    each table is 25 big constant-stationary matmuls; the ACT evacuation
    fuses the b1 bias; one xbar transpose restores node-major rows.
    """
    import concourse.bacc as bacc
    import concourse.mybir as mybir
    import concourse.tile as tile

    f32 = mybir.dt.float32
    fp16 = mybir.dt.float16
    ACT = mybir.ActivationFunctionType

    BLK = 512
    blocks = [(i * BLK, min(BLK, SHARD - i * BLK))
              for i in range(-(-SHARD // BLK))]

    nc = bacc.Bacc("TRN2", target_bir_lowering=False, debug=False,
                   num_devices=N_CORES)
    ushT = nc.dram_tensor("ushT", [128, SHARD], fp16, kind="ExternalInput")
    mshT = nc.dram_tensor("mshT", [128, SHARD], fp16, kind="ExternalInput")
    wk = nc.dram_tensor("wk", [128, 2 * H], fp16, kind="ExternalInput")
    b1c = nc.dram_tensor("b1c", [128, 1], f32, kind="ExternalInput")
    a16o = nc.dram_tensor("a16o", [SHARD, H], fp16, kind="ExternalOutput")
    b16o = nc.dram_tensor("b16o", [SHARD, H], fp16, kind="ExternalOutput")

    with tile.TileContext(nc) as tc:
        with (
            tc.tile_pool(name="src", bufs=2) as spool,
            tc.tile_pool(name="at", bufs=2) as atpool,
            tc.tile_pool(name="stg", bufs=2) as stpool,
            tc.tile_pool(name="w", bufs=1) as wpool,
            tc.tile_pool(name="pa", bufs=4, space="PSUM") as papool,
        ):
            wsb = wpool.tile([128, 2 * H], fp16, tag="wsb")
            nc.sync.dma_start(wsb[:], wk[:])
            bsb = wpool.tile([128, 1], f32, tag="bsb")
            nc.sync.dma_start(bsb[:], b1c[:])

            def one_table(src_d, out_d, wcol, is_b):
                usbT = spool.tile([128, SHARD], fp16, tag="usbT")
                nc.sync.dma_start(usbT[:], src_d[:])
                atT = atpool.tile([128, SHARD], fp16, tag="atT")
                for c0, n in blocks:
                    pa = papool.tile([128, BLK], f32, tag="pa")
                    nc.tensor.matmul(pa[:, :n], wsb[:, wcol * H:(wcol + 1) * H],
                                     usbT[:, c0:c0 + n], start=True, stop=True)
                    if is_b:
                        nc.scalar.activation(atT[:, c0:c0 + n], pa[:, :n],
                                             ACT.Identity, bias=bsb[:],
                                             scale=1.0)
                    else:
                        nc.scalar.activation(atT[:, c0:c0 + n], pa[:, :n],
                                             ACT.Copy)
                stg = stpool.tile([128, NB_A, 128], fp16, tag="stg")
                nc.sync.dma_start_transpose(stg[:], atT[:])
                nc.sync.dma_start(out_d[:], stg[:])

            def body():
                one_table(ushT, a16o, 0, False)
                one_table(mshT, b16o, 1, True)

            body()
            if reps > 1:
                with tc.For_i(0, reps - 1):
                    body()
    nc.compile()
    return nc


def _build_neff_b3(q_cell, pp, reps=1, ablate=()):
    """Chunk-major windowed NEFF with big merged gathers.

    q_cell: tuple of N_CHUNKS*N_WIN ints -- 128-slot quarters per
    (chunk a, window w) cell in a-major order, shared across cores.
    pp: feature split point (positive-sign w2 features first).
    Output: out[s % 128, s // 128] for padded slot s.
    """
    import concourse.bacc as bacc
    import concourse.mybir as mybir
    import concourse.tile as tile

    f32 = mybir.dt.float32
    fp16 = mybir.dt.float16
    i16 = mybir.dt.int16
    ACT = mybir.ActivationFunctionType
    ALU = mybir.AluOpType

    n_q = sum(q_cell)
    s_tot = n_q * 128
    out_cols = n_q

    # window of each quarter (a-major cell order)
    q_win = []
    for ab, q in enumerate(q_cell):
        w = ab % N_WIN
        q_win.extend([w] * q)

    # gather calls: per chunk, quarters merged into <=MAX_CALL-row calls
    chunk_q = [sum(q_cell[a * N_WIN:(a + 1) * N_WIN]) for a in range(N_CHUNKS)]
    calls = []                    # (chunk a, slot0, n_rows)
    q2call = []                   # quarter -> (call idx, slice within call)
    qbase = 0
    for a in range(N_CHUNKS):
        nq_a = chunk_q[a]
        done = 0
        while done < nq_a:
            take = min(MAX_CALL // 128, nq_a - done)
            ci = len(calls)
            calls.append((a, (qbase + done) * 128, take * 128))
            for k in range(take):
                q2call.append((ci, k))
            done += take
        qbase += nq_a
    assert len(q2call) == n_q

    nc = bacc.Bacc("TRN2", target_bir_lowering=False, debug=False,
                   num_devices=N_CORES, num_swdge_queues=N_QUEUES)
    a16 = nc.dram_tensor("a16", [NP, H], fp16, kind="ExternalInput")
    bslab = nc.dram_tensor("bslab", [128, N_WIN * H], fp16, kind="ExternalInput")
    uidx = nc.dram_tensor("uidx", [128, s_tot // 16], i16, kind="ExternalInput")
    dstw = nc.dram_tensor("dstw", [1, s_tot], fp16, kind="ExternalInput")
    wp2 = nc.dram_tensor("wp2", [128, H + 2], f32, kind="ExternalInput")
    out = nc.dram_tensor("out", [128, out_cols], f32, kind="ExternalOutput")

    with tile.TileContext(nc) as tc:
        with (
            tc.tile_pool(name="g", bufs=6) as gpool,
            tc.tile_pool(name="b", bufs=3) as bpool,
            tc.tile_pool(name="s", bufs=3) as spool,
            tc.tile_pool(name="h", bufs=4) as hpool,
            tc.tile_pool(name="ps", bufs=3, space="PSUM") as pspool,
            tc.tile_pool(name="w", bufs=1) as wpool,
            tc.tile_pool(name="o", bufs=1) as opool,
            tc.tile_pool(name="ix", bufs=1) as idxp,
            tc.tile_pool(name="slab", bufs=1) as slabp,
        ):
            uix = idxp.tile([128, s_tot // 16], i16, tag="uix")
            nc.sync.dma_start(uix[:], uidx[:])
            slab = slabp.tile([128, N_WIN * H], fp16, tag="slab")
            nc.sync.dma_start(slab[:], bslab[:])
            wsb = wpool.tile([128, H + 2], f32, tag="wsb")
            nc.sync.dma_start(wsb[:], wp2[:])
            id16 = wpool.tile([128, H], fp16, tag="id16")
            nc.scalar.activation(id16[:], wsb[:, 0:H], ACT.Copy)
            iota = wpool.tile([128, 1], fp16, tag="iota")
            nc.scalar.activation(iota[:], wsb[:, H:H + 1], ACT.Copy)
            b2c = wsb[:, H + 1:H + 2]
            o_rp = opool.tile([128, out_cols], f32, tag="orp")
            o_rm = opool.tile([128, out_cols], f32, tag="orm")
            o_sb = opool.tile([128, out_cols], f32, tag="osb")
            if pp == 128:
                nc.vector.memset(o_rm[:], 0.0)
            if pp == 0:
                nc.vector.memset(o_rp[:], 0.0)

            qctr = [0]

            def body():
                gtiles = [None] * len(calls)
                stile = [None]

                def s_quarter(k):
                    if k % S_GRP == 0:
                        w = min(S_GRP, n_q - k) * 128
                        bco = bpool.tile([128, S_GRP * 128], fp16, tag="b")
                        nc.sync.dma_start(
                            bco[:, :w],
                            dstw[0:1, k * 128:k * 128 + w]
                            .to_broadcast([128, w]))
                        if "iseq" in ablate:
                            return None
                        stile[0] = spool.tile([128, S_GRP, 128], fp16, tag="s",
                                              name="stile")
                        nc.vector.tensor_tensor(
                            stile[0][:, :w // 128, :], bco[:, :w],
                            iota[:].to_broadcast([128, w]),
                            op=ALU.is_equal)
                    if "iseq" in ablate:
                        return None
                    return stile[0][:, k % S_GRP, :]

                def issue_call(ci):
                    if "gather" in ablate:
                        gtiles[ci] = True
                        return
                    a, s0, n = calls[ci]
                    gt = gpool.tile([128, MAX_CALL // 128, H], fp16, tag="ug")
                    nc.gpsimd.dma_gather(
                        gt[:, :n // 128, :], a16[a * CHUNK:(a + 1) * CHUNK, :],
                        uix[:, s0 // 16:(s0 + n) // 16],
                        num_idxs=n, num_idxs_reg=n, elem_size=H,
                        transpose=False, single_packet=False,
                        queue_num=qctr[0] % N_QUEUES)
                    qctr[0] += 1
                    gtiles[ci] = gt

                if "compute" in ablate:
                    for ci in range(len(calls)):
                        issue_call(ci)
                for g0 in range(0, n_q, GRP):
                    if "compute" in ablate:
                        break
                    gn = min(GRP, n_q - g0)
                    # make sure gather tiles for this group's quarters exist
                    for j in range(gn):
                        ci, _ = q2call[g0 + j]
                        if gtiles[ci] is None:
                            issue_call(ci)
                    # prefetch ahead (keeps queues busy)
                    ci_last = q2call[g0 + gn - 1][0]
                    for ahead in (1, 2, 3):
                        cn = ci_last + ahead
                        if cn < len(calls) and gtiles[cn] is None:
                            issue_call(cn)
                    ps = pspool.tile([128, GRP, H], f32, tag="ps")
                    for j in range(gn):
                        q = g0 + j
                        if "mm" in ablate:
                            if "sbuild" not in ablate:
                                s_quarter(q)
                            continue
                        s_ap = s_quarter(q)
                        mini = slab[:, q_win[q] * H:(q_win[q] + 1) * H]
                        nc.tensor.matmul(ps[:, j, :], s_ap, mini,
                                         start=True, stop=False)
                        ci, k = q2call[q]
                        rhs2 = (id16[:] if "gather" in ablate
                                else gtiles[ci][:, k, :])
                        nc.tensor.matmul(ps[:, j, :], id16[:], rhs2,
                                         start=False, stop=True)
                    if "mm" in ablate or "act" in ablate:
                        continue
                    # split relu outputs into two CONTIGUOUS tiles so the
                    # DVE reduces run unstrided at full rate
                    hl = hpool.tile([128, GRP, pp], fp16, tag="hl")
                    hr = hpool.tile([128, GRP, 128 - pp], fp16, tag="hr")
                    if pp > 0:
                        nc.scalar.activation(hl[:, :gn, :], ps[:, :gn, 0:pp],
                                             ACT.Relu)
                    if pp < 128:
                        nc.scalar.activation(hr[:, :gn, :], ps[:, :gn, pp:128],
                                             ACT.Relu)
                    if "reduce" in ablate or "act" in ablate:
                        continue
                    if pp > 0:
                        nc.vector.tensor_reduce(
                            o_rp[:, g0:g0 + gn], hl[:, :gn, :],
                            axis=mybir.AxisListType.X, op=ALU.add)
                    if pp < 128:
                        nc.vector.tensor_reduce(
                            o_rm[:, g0:g0 + gn], hr[:, :gn, :],
                            axis=mybir.AxisListType.X, op=ALU.add)
                # out = rp - rm + b2
                if not ablate:
                    nc.vector.tensor_tensor(o_sb[:], o_rp[:], o_rm[:],
                                            op=ALU.subtract)
                    nc.scalar.activation(o_sb[:], o_sb[:], ACT.Identity,
                                         bias=b2c, scale=1.0)
                else:
                    nc.vector.memset(o_sb[:], 0.0)

            body()
            if reps > 1:
                with tc.For_i(0, reps - 1):
                    body()
            nc.sync.dma_start(out[:], o_sb[:])
    nc.compile()
    return nc, out_cols


def _marshal3(edge_index):
    """dst-sharded, chunk-major (a, w) cell marshalling.

    Core c owns dst rows [c*SHARD, (c+1)*SHARD); its edges are grouped by
    (chunk a = src // CHUNK, window w = (dst % SHARD) // 128) cells in
    a-major order, padded per cell to q_cell[a,w]*128 slots (q_cell
    shared across cores).  Returns q_cell and per-core uidx/sin/inv.
    """
    src = np.asarray(edge_index[0]).astype(np.int64)
    dst = np.asarray(edge_index[1]).astype(np.int64)
    core_of = dst // SHARD
    a_of = src // CHUNK
    n_cells = N_CHUNKS * N_WIN

    # Balanced-window packing: assign each core's 12544 dst rows to its 98
    # windows so per-(chunk, window) edge counts pack tightly under a shared
    # cap grid (minimizing 128-slot padding).  win_of[c] maps local dst row
    # -> window.
    win_of = np.empty((N_CORES, SHARD), dtype=np.int64)
    percell = np.zeros((N_CORES, N_CHUNKS, N_WIN), dtype=np.int64)
    # shared cap grid: X windows at 3 quarters, rest at 2 (per chunk)
    demand = np.zeros((N_CORES, N_CHUNKS), dtype=np.int64)
    rowcnts = []
    for c in range(N_CORES):
        m = core_of == c
        loc = dst[m] - c * SHARD
        rowcnt = np.zeros((SHARD, N_CHUNKS), dtype=np.int64)
        np.add.at(rowcnt, (loc, a_of[m]), 1)
        rowcnts.append(rowcnt)
        demand[c] = rowcnt.sum(axis=0)
    dmax = int(demand.max())
    X = min(N_WIN, max(0, -(-(dmax + 500 - 254 * N_WIN) // 128)))
    for c in range(N_CORES):
        rowcnt = rowcnts[c]
        tot = rowcnt.sum(axis=1)
        order = np.argsort(-tot, kind="stable")
        caps = np.full((N_WIN, N_CHUNKS), 3 * 128 - 1, dtype=np.int64)
        caps[X:] = 2 * 128 - 1
        room = np.full(N_WIN, W_SZ, dtype=np.int64)
        used = np.zeros((N_WIN, N_CHUNKS), dtype=np.int64)
        for r in order:
            v = rowcnt[r]
            ok = (room > 0) & np.all(used + v <= caps, axis=1)
            if ok.any():
                cand = np.nonzero(ok)[0]
                # best fit: window whose max normalized load after placing
                # is largest (fills tight windows first) with room spread
                resid = ((used[cand] + v) / caps[cand]).max(axis=1)
                w = cand[np.argmax(resid - 0.001 * room[cand])]
            else:  # overflow: window with most per-chunk headroom
                cand = np.nonzero(room > 0)[0]
                w = cand[np.argmax((caps[cand] - used[cand] - v).min(axis=1))]
            win_of[c, r] = w
            used[w] += v
            room[w] -= 1
        percell[c] = used.T
    cnt = percell.reshape(N_CORES, n_cells)
    q_cell = tuple(int(x) for x in -(-cnt.max(axis=0) // 128))

    # fall back to the plain row->window layout if packing did not win
    plain_w = np.tile((np.arange(SHARD) // W_SZ), (N_CORES, 1))
    cnt_pl = np.zeros((N_CORES, n_cells), dtype=np.int64)
    for c in range(N_CORES):
        m = core_of == c
        cell_pl = a_of[m] * N_WIN + plain_w[c, dst[m] - c * SHARD]
        cnt_pl[c] = np.bincount(cell_pl, minlength=n_cells)
    q_pl = tuple(int(x) for x in -(-cnt_pl.max(axis=0) // 128))
    if sum(q_pl) <= sum(q_cell):
        win_of = plain_w
        cnt = cnt_pl
        q_cell = q_pl

    w_of = win_of[core_of, dst - core_of * SHARD]
    cell_of = a_of * N_WIN + w_of
    n_q = int(sum(q_cell))
    s_tot = n_q * 128
    cell_base = np.concatenate([[0], np.cumsum(np.asarray(q_cell) * 128)])

    cores = []
    for c in range(N_CORES):
        m = np.nonzero(core_of == c)[0]
        order = m[np.argsort(cell_of[m], kind="stable")]
        cells_sorted = cell_of[order]
        starts = np.searchsorted(cells_sorted, np.arange(n_cells), side="left")
        within = np.arange(order.size) - starts[cells_sorted]
        slots = cell_base[cells_sorted] + within

        # pad slots get spread indices -- a constant (eg row 0) makes every
        # padded descriptor hit one HBM address and halves gather throughput
        uloc = (np.arange(s_tot, dtype=np.int64) * 97 % CHUNK).astype(np.int16)
        uloc[slots] = (src[order] - a_of[order] * CHUNK).astype(np.int16)
        # rank of each dst row within its window (slab row index)
        lw = win_of[c]
        rank = np.zeros(SHARD, dtype=np.int64)
        ws = np.argsort(lw, kind="stable")
        rank[ws] = np.arange(SHARD) - np.searchsorted(lw[ws], lw[ws]) * 0
        starts_w = np.searchsorted(lw[ws], np.arange(N_WIN))
        rank[ws] = np.arange(SHARD) - starts_w[lw[ws]]
        dstrow = np.full(s_tot, -1, dtype=np.int64)
        dstrow[slots] = rank[dst[order] - c * SHARD]
        inv = np.full(s_tot, -1, dtype=np.int64)
        inv[slots] = order

        dstw = np.ascontiguousarray(
            dstrow.astype(np.float16).reshape(1, s_tot))

        wrapped = np.ascontiguousarray(
            np.tile(uloc.reshape(s_tot // 16, 16).T, (8, 1)))
        perm = np.argsort(win_of[c] * SHARD + rank, kind="stable")
        cores.append({"uidx": wrapped, "dstw": dstw, "inv": inv,
                      "rowperm": perm})
    return q_cell, n_q, cores


def _prep_wp2(W2, b2):
    wp2 = np.zeros((128, H + 2), dtype=np.float32)
    wp2[:, 0:H] = np.eye(128, dtype=np.float32)
    wp2[:, H] = np.arange(128, dtype=np.float32)
    wp2[:, H + 1] = np.asarray(b2, dtype=np.float32)[0]
    return wp2


def _slab_for_core(B16, c, rowperm=None):
    rows = B16[c * SHARD:(c + 1) * SHARD]
    if rowperm is not None:
        rows = rows[rowperm]
    return np.ascontiguousarray(
        rows.reshape(N_WIN, W_SZ, H).transpose(1, 0, 2).reshape(128, N_WIN * H))


def _fold_w2(W1, b1, W2):
    """Fold |w2| scaling + positive-first sign permutation into W1/b1."""
    W1 = np.asarray(W1, dtype=np.float32)
    b1 = np.asarray(b1, dtype=np.float32)
    w2 = np.asarray(W2, dtype=np.float32).reshape(H)
    sign_neg = w2 < 0
    perm = np.argsort(sign_neg, kind="stable")  # positives (and 0) first
    pp = int((~sign_neg).sum())
    scale = np.abs(w2)
    W1p = (W1 * scale[None, :])[:, perm]
    b1p = (b1 * scale)[perm]
    return W1p, b1p, pp


def _prep_a_inputs(user_features, movie_features, W1, b1, W2=None):
    if W2 is not None:
        W1, b1, _ = _fold_w2(W1, b1, W2)
    uf = np.zeros((NP, H), dtype=np.float16)
    uf[:N_NODES] = user_features.astype(np.float16)
    mf = np.zeros((NP, H), dtype=np.float16)
    mf[:N_NODES] = movie_features.astype(np.float16)
    wk = np.zeros((128, 2 * H), dtype=np.float16)
    wk[:, 0:H] = np.asarray(W1, dtype=np.float32)[:H].astype(np.float16)
    wk[:, H:2 * H] = np.asarray(W1, dtype=np.float32)[H:].astype(np.float16)
    b1c = np.ascontiguousarray(
        np.asarray(b1, dtype=np.float32).reshape(128, 1))
    j = np.arange(SHARD)
    rowmap = (j % 128) * NB_A + j // 128
    outs = []
    for c in range(N_CORES):
        us = uf[c * SHARD:(c + 1) * SHARD]
        ms = mf[c * SHARD:(c + 1) * SHARD]
        outs.append({"ushT": np.ascontiguousarray(us[rowmap].T),
                     "mshT": np.ascontiguousarray(ms[rowmap].T),
                     "wk": wk, "b1c": b1c})
    return outs


def kernel(user_features, movie_features, edge_index, W1, b1, W2, b2):
    from concourse.bass_utils import run_bass_kernel_spmd

    user_features = np.ascontiguousarray(user_features, dtype=np.float32)
    movie_features = np.ascontiguousarray(movie_features, dtype=np.float32)
    ei = np.ascontiguousarray(edge_index)
    E = ei.shape[1]

    W1p, b1p, pp = _fold_w2(W1, b1, W2)

    # ---- NEFF-A: device-side A = U@W1top', B' = M@W1bot' + b1' (fp16) ----
    if "A" not in _cache:
        _cache["A"] = _build_neff_a()
    nca = _cache["A"]
    in_a = _prep_a_inputs(user_features, movie_features, W1p, b1p)
    res_a = run_bass_kernel_spmd(nca, in_a, core_ids=list(range(N_CORES)))
    A16 = np.concatenate([res_a.results[c]["a16o"] for c in range(N_CORES)])
    B16 = np.concatenate([res_a.results[c]["b16o"] for c in range(N_CORES)])

    # ---- host marshalling of edges (chunk-major windowed cells) ----
    q_cell, n_q, cores = _marshal3(ei)

    key_b = ("B3", q_cell, pp)
    if key_b not in _cache:
        _cache[key_b] = _build_neff_b3(q_cell, pp)
    ncb, out_cols = _cache[key_b]

    wp2 = _prep_wp2(W2, b2)
    in_b = [{"a16": A16,
             "bslab": _slab_for_core(B16, c, cores[c]["rowperm"]),
             "uidx": cores[c]["uidx"], "dstw": cores[c]["dstw"], "wp2": wp2}
            for c in range(N_CORES)]
    res_b = run_bass_kernel_spmd(ncb, in_b, core_ids=list(range(N_CORES)))

    # ---- host inverse permutation ----
    # padded-stream slot s lives at device out[s % 128, s // 128]
    out = np.empty(E, dtype=np.float32)
    s = np.arange(n_q * 128)
    flat_pos = (s % 128) * out_cols + s // 128
    for c in range(N_CORES):
        vals = res_b.results[c]["out"].reshape(-1)[flat_pos]
        inv = cores[c]["inv"]
        mask = inv >= 0
        out[inv[mask]] = vals[mask]
    return out


# revision 22
# speedup vs baseline: 1.3128x; 1.0296x over previous
"""GNN message-passing edge scorer on 8 TRN2 NeuronCores.

Model: out[e] = relu(concat(U[src[e]], M[dst[e]]) @ W1 + b1) @ W2 + b2
  U, M: [100000, 128] f32 node tables; edge_index: [2, 1000000] int32/64.

v5 strategy (edge-parallel, tables replicated; algebraic restructure):
  W2 is folded into the tables on the host: with g = |w2|*(a_src + b_dst)
  in a feature order permuted so positive-sign-w2 features come first,
    out[e] = sum_{f<PP} relu(g_f) - sum_{f>=PP} relu(g_f) + b2.
  - NEFF-A (8-core SPMD): per-core 1/8 shard of the pre-mixed+scaled
    tables A = U @ (W1top |w2| perm), B' = M @ (W1bot |w2| perm) + b1'.
    Host-transposed fp16 inputs -> 25 constant-stationary matmuls per
    table, bias fused in the ACT evacuation, one xbar transpose back to
    node-major fp16 rows.
  - NEFF-B (8-core SPMD): edges sharded by dst core; per-core edges
    grouped into (src-chunk a, dst-window w) cells, chunk-major, each
    padded to 128-slot quarters (shared q_cell across cores).  B' rows
    expanded from the SBUF-resident slab by one-hot S matmuls; S is built
    ON DEVICE (dstrow stream + partition-broadcast DMA + DVE is_equal
    against an iota column).  A rows pulled by dma_gather in big merged
    4096-row calls.  Per 4-quarter group: 4x(S-matmul + gathered-add
    matmul) into one PSUM bank, two ACT relus into CONTIGUOUS sign-split
    tiles, two unstrided DVE tensor_reduces.  End: out = rp - rm + b2.

HW facts this design is built on (measured on this runtime):
  - dma_gather: ~1.9-2.6 ns/row with 4 SWDGE queues and 4096-row calls
    (per-ROW descriptor cost, independent of elem size); ~2.9 at 512-row
    calls; 8192-row calls are SLOWER (ring thrash).  Rates degrade ~20%
    under sustained load (R=65 vs R=9 rep loops).
  - Constant (row-0) padding indices halve gather throughput (single-HBM
    -address hotspot) -- pad slots must use SPREAD indices.
  - num_idxs_reg truncation, trailing -1 indices, and single_packet=True
    all HANG the device => every padded slot must really be gathered.
  - indirect_dma_start: ~11 ns/row -- not competitive.
  - Host-streamed one-hot S (38.5 MB/core) saturates HBM alongside the
    gather traffic; the on-device build keeps S traffic at 0.3 MB.
  - PSUM accumulation groups must be CONSECUTIVE PE instructions --
    interleaving two groups' matmuls corrupts results.
  - 20% cell padding is statistically forced: window-chunk counts are
    ~Poisson(320) and ceil to 3 quarters; balanced bin-packing of dst
    rows into windows cannot create 2-quarter cells.
"""

import numpy as np

N_NODES = 100000
H = 128
N_CORES = 8
SHARD = 12544                 # NEFF-A rows per core (98 * 128)
NB_A = SHARD // 128           # 98 blocks per table per core
NP = SHARD * N_CORES          # 100352 padded table rows
N_CHUNKS = 4
CHUNK = NP // N_CHUNKS        # 25088, int16-addressable
W_SZ = 128                    # dst window rows
N_WIN = SHARD // W_SZ         # 98 windows per core
MAX_CALL = 4096               # indices per dma_gather call
N_QUEUES = 4                  # SWDGE queues for gathers
GRP = 4                       # quarters per compute group
S_GRP = 16                    # S-matrix quarters per streaming DMA

_cache = {}


def _build_neff_a(reps=1):
    """Table precompute: A = U @ W1top', B = M @ W1bot' + b1'.

    Inputs are host-TRANSPOSED fp16 (ushT[:, nb*128+p] = U[p*NB_A+nb]) so
    each table is 25 big constant-stationary matmuls; the ACT evacuation
    fuses the b1 bias; one xbar transpose restores node-major rows.
    """
    import concourse.bacc as bacc
    import concourse.mybir as mybir
    import concourse.tile as tile

    f32 = mybir.dt.float32
    fp16 = mybir.dt.float16
    ACT = mybir.ActivationFunctionType

    BLK = 512
    blocks = [(i * BLK, min(BLK, SHARD - i * BLK))
              for i in range(-(-SHARD // BLK))]

    nc = bacc.Bacc("TRN2", target_bir_lowering=False, debug=False,
                   num_devices=N_CORES)
    ushT = nc.dram_tensor("ushT", [128, SHARD], fp16, kind="ExternalInput")
    mshT = nc.dram_tensor("mshT", [128, SHARD], fp16, kind="ExternalInput")
    wk = nc.dram_tensor("wk", [128, 2 * H], fp16, kind="ExternalInput")
    b1c = nc.dram_tensor("b1c", [128, 1], f32, kind="ExternalInput")
    a16o = nc.dram_tensor("a16o", [SHARD, H], fp16, kind="ExternalOutput")
    b16o = nc.dram_tensor("b16o", [SHARD, H], fp16, kind="ExternalOutput")

    with tile.TileContext(nc) as tc:
        with (
            tc.tile_pool(name="src", bufs=2) as spool,
            tc.tile_pool(name="at", bufs=2) as atpool,
            tc.tile_pool(name="stg", bufs=2) as stpool,
            tc.tile_pool(name="w", bufs=1) as wpool,
            tc.tile_pool(name="pa", bufs=4, space="PSUM") as papool,
        ):
            wsb = wpool.tile([128, 2 * H], fp16, tag="wsb")
            nc.sync.dma_start(wsb[:], wk[:])
            bsb = wpool.tile([128, 1], f32, tag="bsb")
            nc.sync.dma_start(bsb[:], b1c[:])

            def one_table(src_d, out_d, wcol, is_b):
                usbT = spool.tile([128, SHARD], fp16, tag="usbT")
                nc.sync.dma_start(usbT[:], src_d[:])
                atT = atpool.tile([128, SHARD], fp16, tag="atT")
                for c0, n in blocks:
                    pa = papool.tile([128, BLK], f32, tag="pa")
                    nc.tensor.matmul(pa[:, :n], wsb[:, wcol * H:(wcol + 1) * H],
                                     usbT[:, c0:c0 + n], start=True, stop=True)
                    if is_b:
                        nc.scalar.activation(atT[:, c0:c0 + n], pa[:, :n],
                                             ACT.Identity, bias=bsb[:],
                                             scale=1.0)
                    else:
                        nc.scalar.activation(atT[:, c0:c0 + n], pa[:, :n],
                                             ACT.Copy)
                stg = stpool.tile([128, NB_A, 128], fp16, tag="stg")
                nc.sync.dma_start_transpose(stg[:], atT[:])
                nc.sync.dma_start(out_d[:], stg[:])

            def body():
                one_table(ushT, a16o, 0, False)
                one_table(mshT, b16o, 1, True)

            body()
            if reps > 1:
                with tc.For_i(0, reps - 1):
                    body()
    nc.compile()
    return nc


def _build_neff_b3(q_cell, pp, reps=1, ablate=()):
    """Chunk-major windowed NEFF with big merged gathers.

    q_cell: tuple of N_CHUNKS*N_WIN ints -- 128-slot quarters per
    (chunk a, window w) cell in a-major order, shared across cores.
    pp: feature split point (positive-sign w2 features first).
    Output: out[s % 128, s // 128] for padded slot s.
    """
    import concourse.bacc as bacc
    import concourse.mybir as mybir
    import concourse.tile as tile

    f32 = mybir.dt.float32
    fp16 = mybir.dt.float16
    i16 = mybir.dt.int16
    ACT = mybir.ActivationFunctionType
    ALU = mybir.AluOpType

    n_q = sum(q_cell)
    s_tot = n_q * 128
    out_cols = n_q

    # window of each quarter (a-major cell order)
    q_win = []
    for ab, q in enumerate(q_cell):
        w = ab % N_WIN
        q_win.extend([w] * q)

    # gather calls: per chunk, quarters merged into <=MAX_CALL-row calls
    chunk_q = [sum(q_cell[a * N_WIN:(a + 1) * N_WIN]) for a in range(N_CHUNKS)]
    calls = []                    # (chunk a, slot0, n_rows)
    q2call = []                   # quarter -> (call idx, slice within call)
    qbase = 0
    for a in range(N_CHUNKS):
        nq_a = chunk_q[a]
        done = 0
        while done < nq_a:
            take = min(MAX_CALL // 128, nq_a - done)
            ci = len(calls)
            calls.append((a, (qbase + done) * 128, take * 128))
            for k in range(take):
                q2call.append((ci, k))
            done += take
        qbase += nq_a
    assert len(q2call) == n_q

    nc = bacc.Bacc("TRN2", target_bir_lowering=False, debug=False,
                   num_devices=N_CORES, num_swdge_queues=N_QUEUES)
    a16 = nc.dram_tensor("a16", [NP, H], fp16, kind="ExternalInput")
    bslab = nc.dram_tensor("bslab", [128, N_WIN * H], fp16, kind="ExternalInput")
    uidx = nc.dram_tensor("uidx", [128, s_tot // 16], i16, kind="ExternalInput")
    dstw = nc.dram_tensor("dstw", [1, s_tot], fp16, kind="ExternalInput")
    wp2 = nc.dram_tensor("wp2", [128, H + 2], f32, kind="ExternalInput")
    out = nc.dram_tensor("out", [128, out_cols], f32, kind="ExternalOutput")

    with tile.TileContext(nc) as tc:
        with (
            tc.tile_pool(name="g", bufs=6) as gpool,
            tc.tile_pool(name="b", bufs=3) as bpool,
            tc.tile_pool(name="s", bufs=3) as spool,
            tc.tile_pool(name="h", bufs=4) as hpool,
            tc.tile_pool(name="ps", bufs=3, space="PSUM") as pspool,
            tc.tile_pool(name="w", bufs=1) as wpool,
            tc.tile_pool(name="o", bufs=1) as opool,
            tc.tile_pool(name="ix", bufs=1) as idxp,
            tc.tile_pool(name="slab", bufs=1) as slabp,
        ):
            uix = idxp.tile([128, s_tot // 16], i16, tag="uix")
            nc.sync.dma_start(uix[:], uidx[:])
            slab = slabp.tile([128, N_WIN * H], fp16, tag="slab")
            nc.sync.dma_start(slab[:], bslab[:])
            wsb = wpool.tile([128, H + 2], f32, tag="wsb")
            nc.sync.dma_start(wsb[:], wp2[:])
            id16 = wpool.tile([128, H], fp16, tag="id16")
            nc.scalar.activation(id16[:], wsb[:, 0:H], ACT.Copy)
            iota = wpool.tile([128, 1], fp16, tag="iota")
            nc.scalar.activation(iota[:], wsb[:, H:H + 1], ACT.Copy)
            b2c = wsb[:, H + 1:H + 2]
            o_rp = opool.tile([128, out_cols], f32, tag="orp")
            o_rm = opool.tile([128, out_cols], f32, tag="orm")
            o_sb = opool.tile([128, out_cols], f32, tag="osb")
            if pp == 128:
                nc.vector.memset(o_rm[:], 0.0)
            if pp == 0:
                nc.vector.memset(o_rp[:], 0.0)

            qctr = [0]

            def body():
                gtiles = [None] * len(calls)
                stile = [None]

                def s_quarter(k):
                    if k % S_GRP == 0:
                        w = min(S_GRP, n_q - k) * 128
                        bco = bpool.tile([128, S_GRP * 128], fp16, tag="b")
                        nc.sync.dma_start(
                            bco[:, :w],
                            dstw[0:1, k * 128:k * 128 + w]
                            .to_broadcast([128, w]))
                        if "iseq" in ablate:
                            return None
                        stile[0] = spool.tile([128, S_GRP, 128], fp16, tag="s",
                                              name="stile")
                        nc.vector.tensor_tensor(
                            stile[0][:, :w // 128, :], bco[:, :w],
                            iota[:].to_broadcast([128, w]),
                            op=ALU.is_equal)
                    if "iseq" in ablate:
                        return None
                    return stile[0][:, k % S_GRP, :]

                def issue_call(ci):
                    if "gather" in ablate:
                        gtiles[ci] = True
                        return
                    a, s0, n = calls[ci]
                    gt = gpool.tile([128, MAX_CALL // 128, H], fp16, tag="ug")
                    nc.gpsimd.dma_gather(
                        gt[:, :n // 128, :], a16[a * CHUNK:(a + 1) * CHUNK, :],
                        uix[:, s0 // 16:(s0 + n) // 16],
                        num_idxs=n, num_idxs_reg=n, elem_size=H,
                        transpose=False, single_packet=False,
                        queue_num=qctr[0] % N_QUEUES)
                    qctr[0] += 1
                    gtiles[ci] = gt

                if "compute" in ablate:
                    for ci in range(len(calls)):
                        issue_call(ci)
                for g0 in range(0, n_q, GRP):
                    if "compute" in ablate:
                        break
                    gn = min(GRP, n_q - g0)
                    # make sure gather tiles for this group's quarters exist
                    for j in range(gn):
                        ci, _ = q2call[g0 + j]
                        if gtiles[ci] is None:
                            issue_call(ci)
                    # prefetch ahead (keeps queues busy)
                    ci_last = q2call[g0 + gn - 1][0]
                    for ahead in (1, 2, 3):
                        cn = ci_last + ahead
                        if cn < len(calls) and gtiles[cn] is None:
                            issue_call(cn)
                    ps = pspool.tile([128, GRP, H], f32, tag="ps")
                    for j in range(gn):
                        q = g0 + j
                        if "mm" in ablate:
                            if "sbuild" not in ablate:
                                s_quarter(q)
                            continue
                        s_ap = s_quarter(q)
                        mini = slab[:, q_win[q] * H:(q_win[q] + 1) * H]
                        nc.tensor.matmul(ps[:, j, :], s_ap, mini,
                                         start=True, stop=False)
                        ci, k = q2call[q]
                        rhs2 = (id16[:] if "gather" in ablate
                                else gtiles[ci][:, k, :])
                        nc.tensor.matmul(ps[:, j, :], id16[:], rhs2,
                                         start=False, stop=True)
                    if "mm" in ablate or "act" in ablate:
                        continue
                    # split relu outputs into two CONTIGUOUS tiles so the
                    # DVE reduces run unstrided at full rate
                    hl = hpool.tile([128, GRP, pp], fp16, tag="hl")
                    hr = hpool.tile([128, GRP, 128 - pp], fp16, tag="hr")
                    if pp > 0:
                        nc.scalar.activation(hl[:, :gn, :], ps[:, :gn, 0:pp],
                                             ACT.Relu)
                    if pp < 128:
                        nc.scalar.activation(hr[:, :gn, :], ps[:, :gn, pp:128],
                                             ACT.Relu)
                    if "reduce" in ablate or "act" in ablate:
                        continue
                    if pp > 0:
                        nc.vector.tensor_reduce(
                            o_rp[:, g0:g0 + gn], hl[:, :gn, :],
                            axis=mybir.AxisListType.X, op=ALU.add)
                    if pp < 128:
                        nc.vector.tensor_reduce(
                            o_rm[:, g0:g0 + gn], hr[:, :gn, :],
                            axis=mybir.AxisListType.X, op=ALU.add)
                # out = rp - rm + b2
                if not ablate:
                    nc.vector.tensor_tensor(o_sb[:], o_rp[:], o_rm[:],
                                            op=ALU.subtract)
                    nc.scalar.activation(o_sb[:], o_sb[:], ACT.Identity,
                                         bias=b2c, scale=1.0)
                else:
                    nc.vector.memset(o_sb[:], 0.0)

            body()
            if reps > 1:
                with tc.For_i(0, reps - 1):
                    body()
            nc.sync.dma_start(out[:], o_sb[:])
    nc.compile()
    return nc, out_cols


def _marshal3(edge_index):
    """dst-sharded, chunk-major (a, w) cell marshalling.

    Core c owns dst rows [c*SHARD, (c+1)*SHARD); its edges are grouped by
    (chunk a = src // CHUNK, window w = (dst % SHARD) // 128) cells in
    a-major order, padded per cell to q_cell[a,w]*128 slots (q_cell
    shared across cores).  Returns q_cell and per-core uidx/sin/inv.
    """
    src = np.asarray(edge_index[0]).astype(np.int64)
    dst = np.asarray(edge_index[1]).astype(np.int64)
    core_of = dst // SHARD
    a_of = src // CHUNK
    n_cells = N_CHUNKS * N_WIN

    w_of = (dst % SHARD) // W_SZ
    cell_of = a_of * N_WIN + w_of
    cnt = np.zeros((N_CORES, n_cells), dtype=np.int64)
    for c in range(N_CORES):
        m = core_of == c
        cnt[c] = np.bincount(cell_of[m], minlength=n_cells)
    q_cell = tuple(int(x) for x in -(-cnt.max(axis=0) // 128))
    n_q = int(sum(q_cell))
    s_tot = n_q * 128
    cell_base = np.concatenate([[0], np.cumsum(np.asarray(q_cell) * 128)])

    cores = []
    for c in range(N_CORES):
        m = np.nonzero(core_of == c)[0]
        order = m[np.argsort(cell_of[m], kind="stable")]
        cells_sorted = cell_of[order]
        starts = np.searchsorted(cells_sorted, np.arange(n_cells), side="left")
        within = np.arange(order.size) - starts[cells_sorted]
        slots = cell_base[cells_sorted] + within

        # pad slots get spread indices -- a constant (eg row 0) makes every
        # padded descriptor hit one HBM address and halves gather throughput
        uloc = (np.arange(s_tot, dtype=np.int64) * 97 % CHUNK).astype(np.int16)
        uloc[slots] = (src[order] - a_of[order] * CHUNK).astype(np.int16)
        dstrow = np.full(s_tot, -1, dtype=np.int64)
        dstrow[slots] = dst[order] % W_SZ
        inv = np.full(s_tot, -1, dtype=np.int64)
        inv[slots] = order

        dstw = np.ascontiguousarray(
            dstrow.astype(np.float16).reshape(1, s_tot))

        wrapped = np.ascontiguousarray(
            np.tile(uloc.reshape(s_tot // 16, 16).T, (8, 1)))
        cores.append({"uidx": wrapped, "dstw": dstw, "inv": inv,
                      "rowperm": None})
    return q_cell, n_q, cores


def _prep_wp2(W2, b2):
    wp2 = np.zeros((128, H + 2), dtype=np.float32)
    wp2[:, 0:H] = np.eye(128, dtype=np.float32)
    wp2[:, H] = np.arange(128, dtype=np.float32)
    wp2[:, H + 1] = np.asarray(b2, dtype=np.float32)[0]
    return wp2


def _slab_for_core(B16, c, rowperm=None):
    rows = B16[c * SHARD:(c + 1) * SHARD]
    if rowperm is not None:
        rows = rows[rowperm]
    return np.ascontiguousarray(
        rows.reshape(N_WIN, W_SZ, H).transpose(1, 0, 2).reshape(128, N_WIN * H))


def _fold_w2(W1, b1, W2):
    """Fold |w2| scaling + positive-first sign permutation into W1/b1."""
    W1 = np.asarray(W1, dtype=np.float32)
    b1 = np.asarray(b1, dtype=np.float32)
    w2 = np.asarray(W2, dtype=np.float32).reshape(H)
    sign_neg = w2 < 0
    perm = np.argsort(sign_neg, kind="stable")  # positives (and 0) first
    pp = int((~sign_neg).sum())
    scale = np.abs(w2)
    W1p = (W1 * scale[None, :])[:, perm]
    b1p = (b1 * scale)[perm]
    return W1p, b1p, pp


def _prep_a_inputs(user_features, movie_features, W1, b1, W2=None):
    if W2 is not None:
        W1, b1, _ = _fold_w2(W1, b1, W2)
    uf = np.zeros((NP, H), dtype=np.float16)
    uf[:N_NODES] = user_features.astype(np.float16)
    mf = np.zeros((NP, H), dtype=np.float16)
    mf[:N_NODES] = movie_features.astype(np.float16)
    wk = np.zeros((128, 2 * H), dtype=np.float16)
    wk[:, 0:H] = np.asarray(W1, dtype=np.float32)[:H].astype(np.float16)
    wk[:, H:2 * H] = np.asarray(W1, dtype=np.float32)[H:].astype(np.float16)
    b1c = np.ascontiguousarray(
        np.asarray(b1, dtype=np.float32).reshape(128, 1))
    j = np.arange(SHARD)
    rowmap = (j % 128) * NB_A + j // 128
    outs = []
    for c in range(N_CORES):
        us = uf[c * SHARD:(c + 1) * SHARD]
        ms = mf[c * SHARD:(c + 1) * SHARD]
        outs.append({"ushT": np.ascontiguousarray(us[rowmap].T),
                     "mshT": np.ascontiguousarray(ms[rowmap].T),
                     "wk": wk, "b1c": b1c})
    return outs


def kernel(user_features, movie_features, edge_index, W1, b1, W2, b2):
    from concourse.bass_utils import run_bass_kernel_spmd

    user_features = np.ascontiguousarray(user_features, dtype=np.float32)
    movie_features = np.ascontiguousarray(movie_features, dtype=np.float32)
    ei = np.ascontiguousarray(edge_index)
    E = ei.shape[1]

    W1p, b1p, pp = _fold_w2(W1, b1, W2)

    # ---- NEFF-A: device-side A = U@W1top', B' = M@W1bot' + b1' (fp16) ----
    if "A" not in _cache:
        _cache["A"] = _build_neff_a()
    nca = _cache["A"]
    in_a = _prep_a_inputs(user_features, movie_features, W1p, b1p)
    res_a = run_bass_kernel_spmd(nca, in_a, core_ids=list(range(N_CORES)))
    A16 = np.concatenate([res_a.results[c]["a16o"] for c in range(N_CORES)])
    B16 = np.concatenate([res_a.results[c]["b16o"] for c in range(N_CORES)])

    # ---- host marshalling of edges (chunk-major windowed cells) ----
    q_cell, n_q, cores = _marshal3(ei)

    key_b = ("B3", q_cell, pp)
    if key_b not in _cache:
        _cache[key_b] = _build_neff_b3(q_cell, pp)
    ncb, out_cols = _cache[key_b]

    wp2 = _prep_wp2(W2, b2)
    in_b = [{"a16": A16,
             "bslab": _slab_for_core(B16, c, cores[c]["rowperm"]),
             "uidx": cores[c]["uidx"], "dstw": cores[c]["dstw"], "wp2": wp2}
            for c in range(N_CORES)]
    res_b = run_bass_kernel_spmd(ncb, in_b, core_ids=list(range(N_CORES)))

    # ---- host inverse permutation ----
    # padded-stream slot s lives at device out[s % 128, s // 128]
    out = np.empty(E, dtype=np.float32)
    s = np.arange(n_q * 128)
    flat_pos = (s % 128) * out_cols + s // 128
    for c in range(N_CORES):
        vals = res_b.results[c]["out"].reshape(-1)[flat_pos]
        inv = cores[c]["inv"]
        mask = inv >= 0
        out[inv[mask]] = vals[mask]
    return out


# revision 23
# speedup vs baseline: 1.4320x; 1.0909x over previous
"""GNN message-passing edge scorer on 8 TRN2 NeuronCores.

Model: out[e] = relu(concat(U[src[e]], M[dst[e]]) @ W1 + b1) @ W2 + b2
  U, M: [100000, 128] f32 node tables; edge_index: [2, 1000000] int32/64.

v5 strategy (edge-parallel, tables replicated; algebraic restructure):
  W2 is folded into the tables on the host: with g = |w2|*(a_src + b_dst)
  in a feature order permuted so positive-sign-w2 features come first,
    out[e] = sum_{f<PP} relu(g_f) - sum_{f>=PP} relu(g_f) + b2.
  - NEFF-A (8-core SPMD): per-core 1/8 shard of the pre-mixed+scaled
    tables A = U @ (W1top |w2| perm), B' = M @ (W1bot |w2| perm) + b1'.
    Host-transposed fp16 inputs -> 25 constant-stationary matmuls per
    table, bias fused in the ACT evacuation, one xbar transpose back to
    node-major fp16 rows.
  - NEFF-B (8-core SPMD): edges sharded by dst core; per-core edges
    grouped into (src-chunk a, dst-window w) cells, chunk-major, each
    padded to 128-slot quarters (shared q_cell across cores).  B' rows
    expanded from the SBUF-resident slab by one-hot S matmuls; S is built
    ON DEVICE (dstrow stream + partition-broadcast DMA + DVE is_equal
    against an iota column).  A rows pulled by dma_gather in big merged
    4096-row calls.  Per 4-quarter group: 4x(S-matmul + gathered-add
    matmul) into one PSUM bank, two ACT relus into CONTIGUOUS sign-split
    tiles, two unstrided DVE tensor_reduces.  End: out = rp - rm + b2.

HW facts this design is built on (measured on this runtime):
  - dma_gather: ~1.9-2.6 ns/row with 4 SWDGE queues and 4096-row calls
    (per-ROW descriptor cost, independent of elem size); ~2.9 at 512-row
    calls; 8192-row calls are SLOWER (ring thrash).  Rates degrade ~20%
    under sustained load (R=65 vs R=9 rep loops).
  - Constant (row-0) padding indices halve gather throughput (single-HBM
    -address hotspot) -- pad slots must use SPREAD indices.
  - num_idxs_reg truncation, trailing -1 indices, and single_packet=True
    all HANG the device => every padded slot must really be gathered.
  - indirect_dma_start: ~11 ns/row -- not competitive.
  - Host-streamed one-hot S (38.5 MB/core) saturates HBM alongside the
    gather traffic; the on-device build keeps S traffic at 0.3 MB.
  - PSUM accumulation groups must be CONSECUTIVE PE instructions --
    interleaving two groups' matmuls corrupts results.
  - 20% cell padding is statistically forced: window-chunk counts are
    ~Poisson(320) and ceil to 3 quarters; balanced bin-packing of dst
    rows into windows cannot create 2-quarter cells.
"""

import numpy as np

N_NODES = 100000
H = 128
N_CORES = 8
SHARD = 12544                 # NEFF-A rows per core (98 * 128)
NB_A = SHARD // 128           # 98 blocks per table per core
NP = SHARD * N_CORES          # 100352 padded table rows
N_CHUNKS = 4
CHUNK = NP // N_CHUNKS        # 25088, int16-addressable
W_SZ = 128                    # dst window rows
N_WIN = SHARD // W_SZ         # 98 windows per core
MAX_CALL = 4096               # indices per dma_gather call
N_QUEUES = 4                  # SWDGE queues for gathers
GRP = 8                       # quarters per compute group
S_GRP = 16                    # S-matrix quarters per streaming DMA

_cache = {}


def _build_neff_a(reps=1):
    """Table precompute: A = U @ W1top', B = M @ W1bot' + b1'.

    Inputs are host-TRANSPOSED fp16 (ushT[:, nb*128+p] = U[p*NB_A+nb]) so
    each table is 25 big constant-stationary matmuls; the ACT evacuation
    fuses the b1 bias; one xbar transpose restores node-major rows.
    """
    import concourse.bacc as bacc
    import concourse.mybir as mybir
    import concourse.tile as tile

    f32 = mybir.dt.float32
    fp16 = mybir.dt.float16
    ACT = mybir.ActivationFunctionType

    BLK = 512
    blocks = [(i * BLK, min(BLK, SHARD - i * BLK))
              for i in range(-(-SHARD // BLK))]

    nc = bacc.Bacc("TRN2", target_bir_lowering=False, debug=False,
                   num_devices=N_CORES)
    ushT = nc.dram_tensor("ushT", [128, SHARD], fp16, kind="ExternalInput")
    mshT = nc.dram_tensor("mshT", [128, SHARD], fp16, kind="ExternalInput")
    wk = nc.dram_tensor("wk", [128, 2 * H], fp16, kind="ExternalInput")
    b1c = nc.dram_tensor("b1c", [128, 1], f32, kind="ExternalInput")
    a16o = nc.dram_tensor("a16o", [SHARD, H], fp16, kind="ExternalOutput")
    b16o = nc.dram_tensor("b16o", [SHARD, H], fp16, kind="ExternalOutput")

    with tile.TileContext(nc) as tc:
        with (
            tc.tile_pool(name="src", bufs=2) as spool,
            tc.tile_pool(name="at", bufs=2) as atpool,
            tc.tile_pool(name="stg", bufs=2) as stpool,
            tc.tile_pool(name="w", bufs=1) as wpool,
            tc.tile_pool(name="pa", bufs=4, space="PSUM") as papool,
        ):
            wsb = wpool.tile([128, 2 * H], fp16, tag="wsb")
            nc.sync.dma_start(wsb[:], wk[:])
            bsb = wpool.tile([128, 1], f32, tag="bsb")
            nc.sync.dma_start(bsb[:], b1c[:])

            def one_table(src_d, out_d, wcol, is_b):
                usbT = spool.tile([128, SHARD], fp16, tag="usbT")
                nc.sync.dma_start(usbT[:], src_d[:])
                atT = atpool.tile([128, SHARD], fp16, tag="atT")
                for c0, n in blocks:
                    pa = papool.tile([128, BLK], f32, tag="pa")
                    nc.tensor.matmul(pa[:, :n], wsb[:, wcol * H:(wcol + 1) * H],
                                     usbT[:, c0:c0 + n], start=True, stop=True)
                    if is_b:
                        nc.scalar.activation(atT[:, c0:c0 + n], pa[:, :n],
                                             ACT.Identity, bias=bsb[:],
                                             scale=1.0)
                    else:
                        nc.scalar.activation(atT[:, c0:c0 + n], pa[:, :n],
                                             ACT.Copy)
                stg = stpool.tile([128, NB_A, 128], fp16, tag="stg")
                nc.sync.dma_start_transpose(stg[:], atT[:])
                nc.sync.dma_start(out_d[:], stg[:])

            def body():
                one_table(ushT, a16o, 0, False)
                one_table(mshT, b16o, 1, True)

            body()
            if reps > 1:
                with tc.For_i(0, reps - 1):
                    body()
    nc.compile()
    return nc


def _build_neff_b3(q_cell, pp, reps=1, ablate=()):
    """Chunk-major windowed NEFF with big merged gathers.

    q_cell: tuple of N_CHUNKS*N_WIN ints -- 128-slot quarters per
    (chunk a, window w) cell in a-major order, shared across cores.
    pp: feature split point (positive-sign w2 features first).
    Output: out[s % 128, s // 128] for padded slot s.
    """
    import concourse.bacc as bacc
    import concourse.mybir as mybir
    import concourse.tile as tile

    f32 = mybir.dt.float32
    fp16 = mybir.dt.float16
    i16 = mybir.dt.int16
    ACT = mybir.ActivationFunctionType
    ALU = mybir.AluOpType

    n_q = sum(q_cell)
    s_tot = n_q * 128
    out_cols = n_q

    # window of each quarter (a-major cell order)
    q_win = []
    for ab, q in enumerate(q_cell):
        w = ab % N_WIN
        q_win.extend([w] * q)

    # gather calls: per chunk, quarters merged into <=MAX_CALL-row calls
    chunk_q = [sum(q_cell[a * N_WIN:(a + 1) * N_WIN]) for a in range(N_CHUNKS)]
    calls = []                    # (chunk a, slot0, n_rows)
    q2call = []                   # quarter -> (call idx, slice within call)
    qbase = 0
    for a in range(N_CHUNKS):
        nq_a = chunk_q[a]
        done = 0
        while done < nq_a:
            take = min(MAX_CALL // 128, nq_a - done)
            ci = len(calls)
            calls.append((a, (qbase + done) * 128, take * 128))
            for k in range(take):
                q2call.append((ci, k))
            done += take
        qbase += nq_a
    assert len(q2call) == n_q

    nc = bacc.Bacc("TRN2", target_bir_lowering=False, debug=False,
                   num_devices=N_CORES, num_swdge_queues=N_QUEUES)
    a16 = nc.dram_tensor("a16", [NP, H], fp16, kind="ExternalInput")
    bslab = nc.dram_tensor("bslab", [128, N_WIN * H], fp16, kind="ExternalInput")
    uidx = nc.dram_tensor("uidx", [128, s_tot // 16], i16, kind="ExternalInput")
    dstw = nc.dram_tensor("dstw", [1, s_tot], fp16, kind="ExternalInput")
    wp2 = nc.dram_tensor("wp2", [128, H + 2], f32, kind="ExternalInput")
    outp = nc.dram_tensor("outp", [128, out_cols], f32, kind="ExternalOutput")
    outm = nc.dram_tensor("outm", [128, out_cols], f32, kind="ExternalOutput")

    with tile.TileContext(nc) as tc:
        with (
            tc.tile_pool(name="g", bufs=6) as gpool,
            tc.tile_pool(name="b", bufs=3) as bpool,
            tc.tile_pool(name="s", bufs=3) as spool,
            tc.tile_pool(name="h", bufs=4) as hpool,
            tc.tile_pool(name="ps", bufs=3, space="PSUM") as pspool,
            tc.tile_pool(name="w", bufs=1) as wpool,
            tc.tile_pool(name="o", bufs=1) as opool,
            tc.tile_pool(name="ix", bufs=1) as idxp,
            tc.tile_pool(name="slab", bufs=1) as slabp,
        ):
            uix = idxp.tile([128, s_tot // 16], i16, tag="uix")
            nc.sync.dma_start(uix[:], uidx[:])
            slab = slabp.tile([128, N_WIN * H], fp16, tag="slab")
            nc.sync.dma_start(slab[:], bslab[:])
            wsb = wpool.tile([128, H + 2], f32, tag="wsb")
            nc.sync.dma_start(wsb[:], wp2[:])
            id16 = wpool.tile([128, H], fp16, tag="id16")
            nc.scalar.activation(id16[:], wsb[:, 0:H], ACT.Copy)
            iota = wpool.tile([128, 1], fp16, tag="iota")
            nc.scalar.activation(iota[:], wsb[:, H:H + 1], ACT.Copy)
            o_rp = opool.tile([128, out_cols], f32, tag="orp")
            o_rm = opool.tile([128, out_cols], f32, tag="orm")
            if pp == 128:
                nc.vector.memset(o_rm[:], 0.0)
            if pp == 0:
                nc.vector.memset(o_rp[:], 0.0)

            qctr = [0]

            def body():
                gtiles = [None] * len(calls)
                stile = [None]

                def s_quarter(k):
                    if k % S_GRP == 0:
                        w = min(S_GRP, n_q - k) * 128
                        bco = bpool.tile([128, S_GRP * 128], fp16, tag="b")
                        nc.sync.dma_start(
                            bco[:, :w],
                            dstw[0:1, k * 128:k * 128 + w]
                            .to_broadcast([128, w]))
                        if "iseq" in ablate:
                            return None
                        stile[0] = spool.tile([128, S_GRP, 128], fp16, tag="s",
                                              name="stile")
                        nc.vector.tensor_tensor(
                            stile[0][:, :w // 128, :], bco[:, :w],
                            iota[:].to_broadcast([128, w]),
                            op=ALU.is_equal)
                    if "iseq" in ablate:
                        return None
                    return stile[0][:, k % S_GRP, :]

                def issue_call(ci):
                    if "gather" in ablate:
                        gtiles[ci] = True
                        return
                    a, s0, n = calls[ci]
                    gt = gpool.tile([128, MAX_CALL // 128, H], fp16, tag="ug")
                    nc.gpsimd.dma_gather(
                        gt[:, :n // 128, :], a16[a * CHUNK:(a + 1) * CHUNK, :],
                        uix[:, s0 // 16:(s0 + n) // 16],
                        num_idxs=n, num_idxs_reg=n, elem_size=H,
                        transpose=False, single_packet=False,
                        queue_num=qctr[0] % N_QUEUES)
                    qctr[0] += 1
                    gtiles[ci] = gt

                if "compute" in ablate:
                    for ci in range(len(calls)):
                        issue_call(ci)
                for g0 in range(0, n_q, GRP):
                    if "compute" in ablate:
                        break
                    gn = min(GRP, n_q - g0)
                    # make sure gather tiles for this group's quarters exist
                    for j in range(gn):
                        ci, _ = q2call[g0 + j]
                        if gtiles[ci] is None:
                            issue_call(ci)
                    # prefetch ahead (keeps queues busy)
                    ci_last = q2call[g0 + gn - 1][0]
                    for ahead in (1, 2, 3):
                        cn = ci_last + ahead
                        if cn < len(calls) and gtiles[cn] is None:
                            issue_call(cn)
                    ps = pspool.tile([128, GRP, H], f32, tag="ps")
                    for j in range(gn):
                        q = g0 + j
                        if "mm" in ablate:
                            if "sbuild" not in ablate:
                                s_quarter(q)
                            continue
                        s_ap = s_quarter(q)
                        mini = slab[:, q_win[q] * H:(q_win[q] + 1) * H]
                        nc.tensor.matmul(ps[:, j, :], s_ap, mini,
                                         start=True, stop=False)
                        ci, k = q2call[q]
                        rhs2 = (id16[:] if "gather" in ablate
                                else gtiles[ci][:, k, :])
                        nc.tensor.matmul(ps[:, j, :], id16[:], rhs2,
                                         start=False, stop=True)
                    if "mm" in ablate or "act" in ablate:
                        continue
                    # split relu outputs into two CONTIGUOUS tiles so the
                    # DVE reduces run unstrided at full rate
                    hl = hpool.tile([128, GRP, pp], fp16, tag="hl")
                    hr = hpool.tile([128, GRP, 128 - pp], fp16, tag="hr")
                    if pp > 0:
                        nc.scalar.activation(hl[:, :gn, :], ps[:, :gn, 0:pp],
                                             ACT.Relu)
                    if pp < 128:
                        nc.scalar.activation(hr[:, :gn, :], ps[:, :gn, pp:128],
                                             ACT.Relu)
                    if "reduce" in ablate or "act" in ablate:
                        continue
                    if pp > 0:
                        nc.vector.tensor_reduce(
                            o_rp[:, g0:g0 + gn], hl[:, :gn, :],
                            axis=mybir.AxisListType.X, op=ALU.add)
                    if pp < 128:
                        nc.vector.tensor_reduce(
                            o_rm[:, g0:g0 + gn], hr[:, :gn, :],
                            axis=mybir.AxisListType.X, op=ALU.add)
                if ablate:
                    nc.vector.memset(o_rp[:], 0.0)
                    nc.vector.memset(o_rm[:], 0.0)

            body()
            if reps > 1:
                with tc.For_i(0, reps - 1):
                    body()
            nc.sync.dma_start(outp[:], o_rp[:])
            nc.sync.dma_start(outm[:], o_rm[:])
    nc.compile()
    return nc, out_cols


def _marshal3(edge_index):
    """dst-sharded, chunk-major (a, w) cell marshalling.

    Core c owns dst rows [c*SHARD, (c+1)*SHARD); its edges are grouped by
    (chunk a = src // CHUNK, window w = (dst % SHARD) // 128) cells in
    a-major order, padded per cell to q_cell[a,w]*128 slots (q_cell
    shared across cores).  Returns q_cell and per-core uidx/sin/inv.
    """
    src = np.asarray(edge_index[0]).astype(np.int64)
    dst = np.asarray(edge_index[1]).astype(np.int64)
    core_of = dst // SHARD
    a_of = src // CHUNK
    n_cells = N_CHUNKS * N_WIN

    w_of = (dst % SHARD) // W_SZ
    cell_of = a_of * N_WIN + w_of
    cnt = np.zeros((N_CORES, n_cells), dtype=np.int64)
    for c in range(N_CORES):
        m = core_of == c
        cnt[c] = np.bincount(cell_of[m], minlength=n_cells)
    q_cell = tuple(int(x) for x in -(-cnt.max(axis=0) // 128))
    n_q = int(sum(q_cell))
    s_tot = n_q * 128
    cell_base = np.concatenate([[0], np.cumsum(np.asarray(q_cell) * 128)])

    cores = []
    for c in range(N_CORES):
        m = np.nonzero(core_of == c)[0]
        order = m[np.argsort(cell_of[m], kind="stable")]
        cells_sorted = cell_of[order]
        starts = np.searchsorted(cells_sorted, np.arange(n_cells), side="left")
        within = np.arange(order.size) - starts[cells_sorted]
        slots = cell_base[cells_sorted] + within

        # pad slots get spread indices -- a constant (eg row 0) makes every
        # padded descriptor hit one HBM address and halves gather throughput
        uloc = (np.arange(s_tot, dtype=np.int64) * 97 % CHUNK).astype(np.int16)
        uloc[slots] = (src[order] - a_of[order] * CHUNK).astype(np.int16)
        dstrow = np.full(s_tot, -1, dtype=np.int64)
        dstrow[slots] = dst[order] % W_SZ
        inv = np.full(s_tot, -1, dtype=np.int64)
        inv[slots] = order

        dstw = np.ascontiguousarray(
            dstrow.astype(np.float16).reshape(1, s_tot))

        wrapped = np.ascontiguousarray(
            np.tile(uloc.reshape(s_tot // 16, 16).T, (8, 1)))
        cores.append({"uidx": wrapped, "dstw": dstw, "inv": inv,
                      "rowperm": None})
    return q_cell, n_q, cores


def _prep_wp2(W2, b2):
    wp2 = np.zeros((128, H + 2), dtype=np.float32)
    wp2[:, 0:H] = np.eye(128, dtype=np.float32)
    wp2[:, H] = np.arange(128, dtype=np.float32)
    wp2[:, H + 1] = np.asarray(b2, dtype=np.float32)[0]
    return wp2


def _slab_for_core(B16, c, rowperm=None):
    rows = B16[c * SHARD:(c + 1) * SHARD]
    if rowperm is not None:
        rows = rows[rowperm]
    return np.ascontiguousarray(
        rows.reshape(N_WIN, W_SZ, H).transpose(1, 0, 2).reshape(128, N_WIN * H))


def _fold_w2(W1, b1, W2):
    """Fold |w2| scaling + positive-first sign permutation into W1/b1."""
    W1 = np.asarray(W1, dtype=np.float32)
    b1 = np.asarray(b1, dtype=np.float32)
    w2 = np.asarray(W2, dtype=np.float32).reshape(H)
    sign_neg = w2 < 0
    perm = np.argsort(sign_neg, kind="stable")  # positives (and 0) first
    pp = int((~sign_neg).sum())
    scale = np.abs(w2)
    W1p = (W1 * scale[None, :])[:, perm]
    b1p = (b1 * scale)[perm]
    return W1p, b1p, pp


def _prep_a_inputs(user_features, movie_features, W1, b1, W2=None):
    if W2 is not None:
        W1, b1, _ = _fold_w2(W1, b1, W2)
    uf = np.zeros((NP, H), dtype=np.float16)
    uf[:N_NODES] = user_features.astype(np.float16)
    mf = np.zeros((NP, H), dtype=np.float16)
    mf[:N_NODES] = movie_features.astype(np.float16)
    wk = np.zeros((128, 2 * H), dtype=np.float16)
    wk[:, 0:H] = np.asarray(W1, dtype=np.float32)[:H].astype(np.float16)
    wk[:, H:2 * H] = np.asarray(W1, dtype=np.float32)[H:].astype(np.float16)
    b1c = np.ascontiguousarray(
        np.asarray(b1, dtype=np.float32).reshape(128, 1))
    j = np.arange(SHARD)
    rowmap = (j % 128) * NB_A + j // 128
    outs = []
    for c in range(N_CORES):
        us = uf[c * SHARD:(c + 1) * SHARD]
        ms = mf[c * SHARD:(c + 1) * SHARD]
        outs.append({"ushT": np.ascontiguousarray(us[rowmap].T),
                     "mshT": np.ascontiguousarray(ms[rowmap].T),
                     "wk": wk, "b1c": b1c})
    return outs


def kernel(user_features, movie_features, edge_index, W1, b1, W2, b2):
    from concourse.bass_utils import run_bass_kernel_spmd

    user_features = np.ascontiguousarray(user_features, dtype=np.float32)
    movie_features = np.ascontiguousarray(movie_features, dtype=np.float32)
    ei = np.ascontiguousarray(edge_index)
    E = ei.shape[1]

    W1p, b1p, pp = _fold_w2(W1, b1, W2)

    # ---- NEFF-A: device-side A = U@W1top', B' = M@W1bot' + b1' (fp16) ----
    if "A" not in _cache:
        _cache["A"] = _build_neff_a()
    nca = _cache["A"]
    in_a = _prep_a_inputs(user_features, movie_features, W1p, b1p)
    res_a = run_bass_kernel_spmd(nca, in_a, core_ids=list(range(N_CORES)))
    A16 = np.concatenate([res_a.results[c]["a16o"] for c in range(N_CORES)])
    B16 = np.concatenate([res_a.results[c]["b16o"] for c in range(N_CORES)])

    # ---- host marshalling of edges (chunk-major windowed cells) ----
    q_cell, n_q, cores = _marshal3(ei)

    key_b = ("B3", q_cell, pp)
    if key_b not in _cache:
        _cache[key_b] = _build_neff_b3(q_cell, pp)
    ncb, out_cols = _cache[key_b]

    wp2 = _prep_wp2(W2, b2)
    in_b = [{"a16": A16,
             "bslab": _slab_for_core(B16, c, cores[c]["rowperm"]),
             "uidx": cores[c]["uidx"], "dstw": cores[c]["dstw"], "wp2": wp2}
            for c in range(N_CORES)]
    res_b = run_bass_kernel_spmd(ncb, in_b, core_ids=list(range(N_CORES)))

    # ---- host inverse permutation ----
    # padded-stream slot s lives at device out[s % 128, s // 128]
    out = np.empty(E, dtype=np.float32)
    s = np.arange(n_q * 128)
    flat_pos = (s % 128) * out_cols + s // 128
    b2f = np.float32(np.asarray(b2, dtype=np.float32).reshape(-1)[0])
    for c in range(N_CORES):
        rc = res_b.results[c]
        vals = (rc["outp"].reshape(-1)[flat_pos]
                - rc["outm"].reshape(-1)[flat_pos] + b2f)
        inv = cores[c]["inv"]
        mask = inv >= 0
        out[inv[mask]] = vals[mask]
    return out
